# revision 10
# baseline (speedup 1.0000x reference)
"""DirectVoxGO render kernel for 8x TRN2 NeuronCores (Bass/Tile), v3.

Strategy (data-parallel over rays; grids+MLP replicated; host does the
irregular gather, device does all dense math):

 - Host: trilinear corner gather with the 8 trilinear weights PREMULTIPLIED
   into the gathered channel blocks, laid out FEATURE-major:
     G [99, npts] bf16, rows = [8 corners x 9 k0view-channels (72),
                                vd (3), sin(emb) (12), cos(emb) (24:  12)]
   The 8-corner combine *and* the W0 contraction then collapse into ONE
   matmul per 512 points: lhsT = [W0_k0view replicated 8x; W0_viewdir]
   ([99,128] bf16).  No PE transposes, no on-device trilinear, no on-device
   sin/cos (host computes the 27 per-ray embedding rows exactly once per ray
   and expands via ray_id).
 - Density rides a small point-major tensor d8 [P,F,8] bf16 (premultiplied
   corner densities); device sums 8 -> dens plane, softplus -> log1ma.
 - Device per 512-pt group: mm(h1) -> ACT relu+b0 -> mm(h2) -> Pool relu+b1
   -> mm(rgb logits [3,512]) -> DMA straight to DRAM.  Engines: PE 3
   512-streams/group, ACT one pass, Pool one pass, DVE nearly free.
 - Pass 2 (unchanged math): per-ray cumsum of log1ma via tensor_tensor_scan
   + cross-partition carries; scan-min trick broadcasts each ray's start;
   dumps w = T*alpha [P,F] and t2 = log_t+log1ma [P,F1].
 - Host: sigmoid(logits + diffuse), per-ray reduceat of w*rgb, + alphainv.
   (diffuse = 8-corner sum of premultiplied diffuse channels, done host-side;
   b2 folded in since trilinear weights sum to 1.)
"""

import numpy as np

P = 128
GRID = 160
ALPHA_INIT = 0.01
ACT_SHIFT = float(np.log(1.0 / (1.0 - ALPHA_INIT) - 1.0))
DELTA = 0.5
N_RAYS = 8192
N_CORES = 8
BIG = 1.0e30
KROWS = 99          # 72 k0view + 27 viewdir-embedding rows

_BUILD_CACHE = {}


def build_bass(F=1632, J=48, split_waits=True):
    import concourse.bass as bass
    import concourse.mybir as mybir
    from concourse.tile import TileContext
    from concourse.masks import make_identity

    dt = mybir.dt
    Alu = mybir.AluOpType
    Act = mybir.ActivationFunctionType

    F1 = F + 1
    CJ = J * P                  # G columns per chunk
    NG = CJ // 512              # MLP groups per chunk
    assert F % J == 0 and CJ % 512 == 0
    CAP = P * F

    nc = bass.Bass()

    NT = CAP // 1536            # rgb triad-blocks (3 groups of 512 pts each)

    g_h = nc.dram_tensor("g_pm", [KROWS, CAP], dt.bfloat16, kind="ExternalInput")
    d8_h = nc.dram_tensor("d8_pm", [P, F, 8], dt.bfloat16, kind="ExternalInput")
    mask_h = nc.dram_tensor("mask_pm", [P, F], dt.uint8, kind="ExternalInput")
    w0f_h = nc.dram_tensor("W0f", [KROWS, 128], dt.bfloat16, kind="ExternalInput")
    w1_h = nc.dram_tensor("W1b", [128, 128], dt.bfloat16, kind="ExternalInput")
    w2_h = nc.dram_tensor("W2b", [128, 3], dt.bfloat16, kind="ExternalInput")
    b0_h = nc.dram_tensor("b0c", [128, 1], dt.float32, kind="ExternalInput")
    b1_h = nc.dram_tensor("b1c", [128, 1], dt.float32, kind="ExternalInput")
    # rgb logits packed as [9, NT, 512] bf16: row 3*a+ch = channel ch of
    # group a within the triad
    rgb_o = nc.dram_tensor("rgb_out", [9, NT, 512], dt.bfloat16,
                           kind="ExternalOutput")
    w_o = nc.dram_tensor("w_out", [P, F], dt.float32, kind="ExternalOutput")
    t2_o = nc.dram_tensor("t2_out", [P, F1], dt.float32, kind="ExternalOutput")

    with TileContext(nc) as tc:
        with (
            tc.tile_pool(name="const", bufs=1) as cp,
            tc.tile_pool(name="plane", bufs=1) as pl,
        ):
            ident = cp.tile([P, P], dt.float32, tag="ident")
            make_identity(nc, ident[:])
            w0f_sb = cp.tile([KROWS, 128], dt.bfloat16, tag="w0f")
            w1_sb = cp.tile([128, 128], dt.bfloat16, tag="w1")
            w2_sb = cp.tile([128, 3], dt.bfloat16, tag="w2")
            b0_sb = cp.tile([128, 1], dt.float32, tag="b0")
            b1_sb = cp.tile([128, 1], dt.float32, tag="b1")
            shift_sb = cp.tile([P, 1], dt.float32, tag="shiftc")
            nc.sync.dma_start(out=w0f_sb[:], in_=w0f_h[:])
            nc.sync.dma_start(out=w1_sb[:], in_=w1_h[:])
            nc.sync.dma_start(out=w2_sb[:], in_=w2_h[:])
            nc.sync.dma_start(out=b0_sb[:], in_=b0_h[:])
            nc.sync.dma_start(out=b1_sb[:], in_=b1_h[:])
            nc.vector.memset(shift_sb[:], ACT_SHIFT)

            d8f = pl.tile([P, F, 8], dt.bfloat16, tag="d8f")
            t4f = pl.tile([P, F, 4], dt.bfloat16, tag="t4f")
            dens = pl.tile([P, F], dt.float32, tag="dens")
            l1ma = pl.tile([P, F], dt.float32, tag="l1ma")
            apl = pl.tile([P, F], dt.float32, tag="apl")
            spl = pl.tile([P, F], dt.float32, tag="spl")
            upl = pl.tile([P, F], dt.float32, tag="upl")
            t2p = pl.tile([P, F1], dt.float32, tag="t2p")
            maskp = pl.tile([P, F], dt.uint8, tag="maskp")

            # ---------------- PASS 1 ----------------
            # density: whole-plane preload + pairwise corner-sum tree
            nc.sync.dma_start(out=d8f[:], in_=d8_h[:])
            d8v = d8f[:].rearrange("p f (a b) -> p f a b", b=2)
            nc.vector.tensor_tensor(
                out=t4f[:], in0=d8v[:, :, :, 0], in1=d8v[:, :, :, 1],
                op=Alu.add)
            t4v = t4f[:].rearrange("p f (a b) -> p f a b", b=2)
            t2f = t4f[:, :, 0:2]
            nc.vector.tensor_tensor(
                out=t2f, in0=t4v[:, :, :, 0], in1=t4v[:, :, :, 1],
                op=Alu.add)
            nc.vector.tensor_tensor(
                out=dens[:], in0=t4f[:, :, 0], in1=t4f[:, :, 1],
                op=Alu.add)

            with (
                tc.tile_pool(name="gio", bufs=2) as gio,
                tc.tile_pool(name="mlp", bufs=3) as mp,
                tc.tile_pool(name="ps1", bufs=2, space="PSUM") as pp1,
                tc.tile_pool(name="ps2", bufs=2, space="PSUM") as pp2,
                tc.tile_pool(name="ps3", bufs=2, space="PSUM") as pp3,
            ):
                for ct in range(F // J):
                    gt = gio.tile([KROWS, CJ], dt.bfloat16, tag="gt")
                    nc.sync.dma_start(out=gt[:], in_=g_h[:, ct * CJ:(ct + 1) * CJ])

                    # MLP: triads of 3 groups x 512 points; the 3 rgb outputs
                    # pack into one PSUM tile at partition bases 0/32/64
                    for q in range(NG // 3):
                        qt = pp3.tile([67, 512], dt.float32, tag="qt")
                        for a in range(3):
                            off = (q * 3 + a) * 512
                            h1p = pp1.tile([128, 512], dt.float32, tag="h1p")
                            nc.tensor.matmul(
                                out=h1p[:], lhsT=w0f_sb[:],
                                rhs=gt[:, off:off + 512],
                                start=True, stop=True)
                            h1 = mp.tile([128, 512], dt.bfloat16, tag="h1")
                            nc.scalar.activation(
                                out=h1[:], in_=h1p[:], func=Act.Relu,
                                bias=b0_sb[:])
                            h2p = pp2.tile([128, 512], dt.float32, tag="h2p")
                            nc.tensor.matmul(
                                out=h2p[:], lhsT=w1_sb[:], rhs=h1[:],
                                start=True, stop=True)
                            h2 = mp.tile([128, 512], dt.bfloat16, tag="h2")
                            # ACT handles relu+bias+cast ~2x faster than DVE
                            nc.scalar.activation(
                                out=h2[:], in_=h2p[:], func=Act.Relu,
                                bias=b1_sb[:])
                            nc.tensor.matmul(
                                out=qt[32 * a:32 * a + 3, :], lhsT=w2_sb[:],
                                rhs=h2[:], start=True, stop=True)
                        stage = mp.tile([67, 512], dt.bfloat16, tag="stage")
                        nc.vector.tensor_copy(out=stage[:], in_=qt[:])
                        tq = ct * (NG // 3) + q
                        for a in range(3):
                            nc.sync.dma_start(
                                out=rgb_o[3 * a:3 * a + 3, tq, :],
                                in_=stage[32 * a:32 * a + 3, :])

            # ---------------- PASS 2 ----------------
            with (
                tc.tile_pool(name="p2", bufs=2) as p2,
                tc.tile_pool(name="p2ps", bufs=2, space="PSUM") as p2p,
            ):
                nc.sync.dma_start(out=maskp[:], in_=mask_h[:])

                # l1ma = -DELTA * softplus(dens + shift)
                nc.scalar.activation(
                    out=l1ma[:], in_=dens[:], func=Act.Exp,
                    bias=shift_sb[:], scale=1.0)
                nc.scalar.activation(
                    out=l1ma[:], in_=l1ma[:], func=Act.Ln, bias=1.0, scale=1.0)
                nc.vector.tensor_scalar(
                    out=l1ma[:], in0=l1ma[:], scalar1=-DELTA,
                    scalar2=None, op0=Alu.mult)

                # c = inclusive scan of l1ma; exclusive carry across partitions
                nc.vector.tensor_tensor_scan(
                    out=apl[:], data0=l1ma[:], data1=l1ma[:],
                    initial=0.0, op0=Alu.add, op1=Alu.bypass)
                totT = p2p.tile([1, P], dt.float32, tag="totT")
                nc.tensor.transpose(
                    out=totT[:], in_=apl[:, F - 1:F], identity=ident[:])
                row = p2.tile([1, P], dt.float32, tag="row")
                nc.vector.tensor_copy(out=row[:], in_=totT[:])
                row2 = p2.tile([1, P], dt.float32, tag="row2")
                nc.vector.tensor_tensor_scan(
                    out=row2[:], data0=row[:], data1=row[:], initial=0.0,
                    op0=Alu.add, op1=Alu.bypass)
                sh = p2.tile([1, P], dt.float32, tag="sh")
                nc.vector.memset(sh[:], 0.0)
                nc.vector.tensor_copy(out=sh[:, 1:P], in_=row2[:, 0:P - 1])
                carT = p2p.tile([P, 1], dt.float32, tag="carT")
                nc.tensor.matmul(
                    out=carT[:], lhsT=sh[:], rhs=ident[0:1, 0:1],
                    start=True, stop=True)
                car = p2.tile([P, 1], dt.float32, tag="car")
                nc.vector.tensor_copy(out=car[:], in_=carT[:])
                nc.vector.tensor_scalar(
                    out=apl[:], in0=apl[:], scalar1=car[:], scalar2=None,
                    op0=Alu.add)

                # exclusive ex = c - l1ma (in place)
                nc.vector.tensor_tensor(
                    out=apl[:], in0=apl[:], in1=l1ma[:], op=Alu.subtract)

                # masked ex -> scan-min -> s (carry with min)
                nc.vector.memset(spl[:], BIG)
                nc.vector.copy_predicated(
                    out=spl[:], mask=maskp[:], data=apl[:])
                nc.vector.tensor_tensor_scan(
                    out=upl[:], data0=spl[:], data1=spl[:], initial=BIG,
                    op0=Alu.min, op1=Alu.bypass)
                totT2 = p2p.tile([1, P], dt.float32, tag="totT")
                nc.tensor.transpose(
                    out=totT2[:], in_=upl[:, F - 1:F], identity=ident[:])
                rowm = p2.tile([1, P], dt.float32, tag="rowm")
                nc.vector.tensor_copy(out=rowm[:], in_=totT2[:])
                rowm2 = p2.tile([1, P], dt.float32, tag="rowm2")
                nc.vector.tensor_tensor_scan(
                    out=rowm2[:], data0=rowm[:], data1=rowm[:], initial=BIG,
                    op0=Alu.min, op1=Alu.bypass)
                shm = p2.tile([1, P], dt.float32, tag="shm")
                nc.vector.memset(shm[:], BIG)
                nc.vector.tensor_copy(out=shm[:, 1:P], in_=rowm2[:, 0:P - 1])
                carTm = p2p.tile([P, 1], dt.float32, tag="carT")
                nc.tensor.matmul(
                    out=carTm[:], lhsT=shm[:], rhs=ident[0:1, 0:1],
                    start=True, stop=True)
                carm = p2.tile([P, 1], dt.float32, tag="carm")
                nc.vector.tensor_copy(out=carm[:], in_=carTm[:])
                nc.vector.tensor_scalar(
                    out=upl[:], in0=upl[:], scalar1=carm[:], scalar2=None,
                    op0=Alu.min)

                # log_t = ex - s (into spl); t2 = log_t + l1ma
                nc.vector.tensor_tensor(
                    out=spl[:], in0=apl[:], in1=upl[:], op=Alu.subtract)
                nc.vector.tensor_tensor(
                    out=t2p[:, 0:F], in0=spl[:], in1=l1ma[:], op=Alu.add)
                nc.vector.memset(t2p[:, F:F1], 0.0)
                nc.sync.dma_start(out=t2_o[:], in_=t2p[:])

                # w = exp(log_t) * (1 - exp(l1ma))
                nc.scalar.activation(
                    out=upl[:], in_=spl[:], func=Act.Exp, bias=0.0, scale=1.0)
                nc.scalar.activation(
                    out=apl[:], in_=l1ma[:], func=Act.Exp, bias=0.0, scale=1.0)
                nc.vector.tensor_scalar(
                    out=apl[:], in0=apl[:], scalar1=-1.0, scalar2=1.0,
                    op0=Alu.mult, op1=Alu.add)
                nc.vector.tensor_tensor(
                    out=upl[:], in0=upl[:], in1=apl[:], op=Alu.mult)
                nc.sync.dma_start(out=w_o[:], in_=upl[:])

    if split_waits:
        _split_multi_waits(nc, mybir)
    return nc


def _split_multi_waits(nc, mybir):
    """The walrus build in this container encodes at most ONE sync-wait per
    instruction. Tile attaches several. Split the extras onto same-engine
    NoOps placed immediately before."""
    n_split = 0
    for fn in nc.m.functions:
        for blk in fn.blocks:
            out = []
            for ins in blk.instructions:
                si = ins.sync_info
                if si is not None and si.on_wait and len(si.on_wait) > 1:
                    waits = list(si.on_wait)
                    for w in waits[:-1]:
                        nop = mybir.InstNoOp(
                            name=nc.get_next_instruction_name(),
                            engine=ins.engine,
                            ins=[], outs=[],
                            sync_info=mybir.SyncInfo(on_wait=[w], on_update=[]),
                        )
                        out.append(nop)
                        n_split += 1
                    ins.sync_info = mybir.SyncInfo(
                        on_wait=[waits[-1]], on_update=list(si.on_update))
                out.append(ins)
            try:
                blk.instructions = out
            except (AttributeError, TypeError):
                blk.instructions[:] = out
    return n_split


def _host_prep(density_grid, k0_grid, xyz, viewdirs, W0, b0, W1, b1, W2, b2,
               ray_id, F, grid, n_cores):
    import ml_dtypes
    bf16 = ml_dtypes.bfloat16
    F1 = F + 1
    CAP = P * F
    RPC = N_RAYS // n_cores

    density_grid = np.asarray(density_grid, np.float32)
    k0_grid = np.asarray(k0_grid, np.float32)
    xyz = np.asarray(xyz, np.float32)
    viewdirs = np.asarray(viewdirs, np.float32)
    W0 = np.asarray(W0, np.float32)
    W1 = np.asarray(W1, np.float32)
    W2 = np.asarray(W2, np.float32)
    b0 = np.asarray(b0, np.float32)
    b1 = np.asarray(b1, np.float32)
    b2 = np.asarray(b2, np.float32)
    ray_id = np.asarray(ray_id, np.int32)
    M = xyz.shape[0]

    # host tables (channel-split for cheap gathers)
    tbl_kview = np.ascontiguousarray(
        np.moveaxis(k0_grid[3:12], 0, -1).reshape(-1, 9))      # [V,9]
    tbl_dif = np.ascontiguousarray(
        np.moveaxis(k0_grid[0:3], 0, -1).reshape(-1, 3))       # [V,3]
    tbl_dens = np.ascontiguousarray(density_grid[0].reshape(-1))  # [V]

    # trilinear positions / weights (fp32, mirrors reference)
    pos = (xyz + np.float32(1.0)) / np.float32(2.0) * np.float32(grid - 1)
    pos = np.clip(pos, 0.0, np.float32(grid - 1))
    i0 = np.clip(np.floor(pos).astype(np.int64), 0, grid - 2)
    fr = (pos - i0.astype(np.float32)).astype(np.float32)
    vbase = (i0[:, 0] * grid + i0[:, 1]) * grid + i0[:, 2]
    omf = np.float32(1.0) - fr
    wx = (omf[:, 0], fr[:, 0])
    wy = (omf[:, 1], fr[:, 1])
    wz = (omf[:, 2], fr[:, 2])
    corner_off = []
    corner_w = []
    for dx in (0, 1):
        for dy in (0, 1):
            for dz in (0, 1):
                corner_off.append(dx * grid * grid + dy * grid + dz)
                corner_w.append(wx[dx] * wy[dy] * wz[dz])

    # per-ray viewdir embedding rows [27, n_rays] (exact fp32 sin/cos)
    freq = (2.0 ** np.arange(4)).astype(np.float32)
    emb = (viewdirs[:, :, None] * freq).reshape(N_RAYS, 12)
    vdsc = np.concatenate(
        [viewdirs, np.sin(emb), np.cos(emb)], axis=1).astype(np.float32)  # [N_RAYS,27]

    # fused first-layer weights [99, 128]
    W0f = np.zeros((KROWS, 128), np.float32)
    for k in range(8):
        W0f[9 * k:9 * k + 9] = W0[0:9]
    W0f[72:99] = W0[9:36]

    # core split at ray boundaries
    starts_g = np.searchsorted(ray_id, np.arange(N_RAYS)).astype(np.int64)
    core_lo = np.searchsorted(ray_id, np.arange(0, N_RAYS + 1, RPC)).astype(np.int64)

    in_maps, host_data = [], []
    for c in range(n_cores):
        lo, hi = int(core_lo[c]), int(core_lo[c + 1])
        npts = hi - lo
        assert npts <= CAP, f"core {c} has {npts} > {CAP} points"
        i_all = np.arange(npts)
        col = (i_all % F) * P + (i_all // F)        # point i -> G column

        G = np.zeros((KROWS, CAP), bf16)
        dif_sum = np.zeros((npts, 3), np.float32)
        d8 = np.zeros((CAP, 8), bf16)
        vb = vbase[lo:hi]
        for k in range(8):
            idx = vb + corner_off[k]
            wk_ = corner_w[k][lo:hi]
            G[9 * k:9 * k + 9, col] = (tbl_kview[idx] * wk_[:, None]).astype(bf16).T
            d8[:npts, k] = (tbl_dens[idx] * wk_).astype(bf16)
            dif_sum += tbl_dif[idx] * wk_[:, None]
        dif_sum += b2[None, :]
        G[72:99, col] = vdsc[ray_id[lo:hi]].T.astype(bf16)

        mask_pm = np.zeros((CAP,), np.uint8)
        rs = starts_g[c * RPC:(c + 1) * RPC] - lo
        ends_g = np.searchsorted(ray_id, np.arange(c * RPC, (c + 1) * RPC),
                                 side="right").astype(np.int64) - lo
        nonempty = ends_g > rs
        mask_pm[rs[nonempty]] = 1
        if npts < CAP:
            mask_pm[npts] = 1

        def flat(i):
            return (i // F) * F1 + (i % F)

        ZERO = F
        idx_end = np.where(nonempty, flat(ends_g - 1), ZERO).astype(np.int64)

        in_maps.append({
            "g_pm": G,
            "d8_pm": d8.reshape(P, F, 8),
            "mask_pm": mask_pm.reshape(P, F),
            "W0f": W0f.astype(bf16),
            "W1b": W1.astype(bf16),
            "W2b": W2.astype(bf16),
            "b0c": b0.reshape(128, 1),
            "b1c": b1.reshape(128, 1),
        })
        host_data.append({
            "lo": lo, "hi": hi, "npts": npts, "col": col,
            "dif": dif_sum, "rs": rs, "nonempty": nonempty,
            "idx_end": idx_end,
        })
    return in_maps, host_data


def kernel(density_grid, k0_grid, xyz, viewdirs, W0, b0, W1, b1, W2, b2, ray_id,
           _trace=False):
    from concourse import bass_utils

    F, J = 1632, 48
    key = (F, J)
    if key not in _BUILD_CACHE:
        _BUILD_CACHE[key] = build_bass(F=F, J=J)
    nc = _BUILD_CACHE[key]

    ray_id = np.asarray(ray_id, np.int32)
    in_maps, host_data = _host_prep(
        density_grid, k0_grid, xyz, viewdirs, W0, b0, W1, b1, W2, b2,
        ray_id, F, GRID, N_CORES)
    res = bass_utils.run_bass_kernel_spmd(
        nc, in_maps, core_ids=list(range(N_CORES)), trace=_trace)

    F1 = F + 1
    outs = []
    for c in range(N_CORES):
        hd = host_data[c]
        npts = hd["npts"]
        rgbl = res.results[c]["rgb_out"]          # [99, NQ, 512] bf16 logits
        wplane = res.results[c]["w_out"].reshape(-1)   # [P*F]
        t2 = res.results[c]["t2_out"].reshape(-1)      # [P*F1]

        col = hd["col"]
        q = col // 1536
        a = (col % 1536) // 512
        idx = col % 512
        logits = rgbl[(3 * a)[:, None] + np.arange(3)[None, :],
                      q[:, None], idx[:, None]].astype(np.float32) + hd["dif"]
        rgb = 1.0 / (1.0 + np.exp(-logits))
        wrgb = rgb * wplane[:npts, None]

        rs = np.minimum(hd["rs"], max(npts - 1, 0)).astype(np.int64)
        if npts > 0:
            sums = np.add.reduceat(wrgb, rs, axis=0)
        else:
            sums = np.zeros((len(rs), 3), np.float32)
        sums[~hd["nonempty"]] = 0.0
        alphainv = np.exp(t2[hd["idx_end"]])
        outs.append((sums + alphainv[:, None]).astype(np.float32))
    out = np.concatenate(outs, axis=0)
    if _trace:
        return out, res
    return out
